# revision 54
# baseline (speedup 1.0000x reference)
"""ETSFormer forward pass on 8 Trainium2 NeuronCores (Bass/Tile).

Data-parallel over batch: 32 samples -> 8 cores x 4 samples, weights
replicated, no collectives. The reference's FFT machinery is computed
exactly without dense FFT matmuls:
  - freq_attention: Cooley-Tukey 1024 = 8x128 factorization. Inner 8-point
    stage = DVE/Pool linear combos of the eight [128,512] z tiles; outer
    128-point stage = single-pass fp32 matmuls contracting over partitions,
    with the twiddles e^{2 pi i v(8m+k1)/1024} folded into 13 per-k1
    stationary [128,128] matrices (fmats/imats). Frequencies are indexed
    f = k1 + 8m, k1 = 0..4 (k1 > 4 via conjugate symmetry); top-4 ranking
    happens over a transposed amp layout [c, (k1,m)] with mirror-duplicate
    zones rebuilt by permutation matmuls so dedup matches the reference's
    513-frequency ranking exactly; the keep-mask applies in [m, c] layout
    and the inverse CT reverses the factorization (combine on DVE/Pool).
  - mhesa / level exponential smoothing: first-order EMA -> hardware
    prefix scan (tensor_tensor_scan); fourier_extrapolate: exact slice.

Precision: the top-4 ranking is extremely sensitive (2e-4 relative amp
noise can flip ranks -> 1e-2-class output error), so the CT forward path
(conv, stage-1 combos, stage-2 matmuls, amp) is exact fp32 in BOTH layers.
fp32r (1 cyc/row vs fp32's 4, ~2e-4 relative truncation) is used for the
mhesa win/wout, FF w1/w2, the inverse CT of both layers, level and output
paths; measured rel err 1.08e-2 vs the 2e-2 gate (deterministic for fixed
inputs). Host-side packing loads each constant group in one DMA (HWDGE
charges 625ns fixed per dma_start, so DMA count dominates transfer cost).
"""
import numpy as np
from contextlib import ExitStack

import concourse.bass as bass
import concourse.bacc as bacc
import concourse.tile as tile
from concourse import mybir
from concourse.bass_utils import run_bass_kernel_spmd

F32 = mybir.dt.float32
F32R = mybir.dt.float32r
BF16 = mybir.dt.bfloat16
AF = mybir.ActivationFunctionType
ALU = mybir.AluOpType

N = 1024
D = 512
TF = 7
HEADS = 8
DH = D // HEADS
L = 2
S = 4
NCORES = 8
HOR = 96
FD = 2048
NT = N // 128   # 8
ND = D // 128   # 4
NM = FD // 128  # 16

_CACHE = {}
OMA_BCAST = True


def _ct_consts():
    """Folded-twiddle CT-DFT matrices.
    fmats [128, 13*128]: [FC0, FS0n | FC1, FS1, FS1n | ... | FC4, FS4n],
      FCk1[v, m] = cos(2 pi v (8m+k1) / 1024), FS = sin, *n = negated.
    imats: inverse, [m, v]-layout, scaled by (1 or 2)/1024.
    pm/jr: permutation matrices for mirror-duplicate zones."""
    if "fm" not in _CACHE:
        v = np.arange(128)
        m = np.arange(128)
        fmats, imats = [], []
        for k1 in range(5):
            th = 2.0 * np.pi * np.outer(v, 8 * m + k1) / N
            FC, FS = np.cos(th), np.sin(th)
            sc = (1.0 if k1 in (0, 4) else 2.0) / N
            thi = 2.0 * np.pi * np.outer(8 * m + k1, v) / N
            IC, IS = np.cos(thi) * sc, np.sin(thi) * sc
            if k1 in (0, 4):
                fmats += [FC, -FS]
                imats += [IC, -IS]
            else:
                fmats += [FC, FS, -FS]
                imats += [IC, IS, -IS]
        _CACHE["fm"] = np.concatenate(fmats, axis=1).astype(np.float32)
        _CACHE["im"] = np.concatenate(imats, axis=1).astype(np.float32)
        pm = np.zeros((128, 128), np.float32)
        pm[(128 - np.arange(128)) % 128, np.arange(128)] = 1.0
        jr = np.zeros((128, 128), np.float32)
        jr[127 - np.arange(128), np.arange(128)] = 1.0
        _CACHE["pm"] = pm
        _CACHE["jr"] = jr
    return _CACHE["fm"], _CACHE["im"], _CACHE["pm"], _CACHE["jr"]


def _sl(i, w=128):
    return slice(i * w, (i + 1) * w)


def _build_w2d(conv_w, conv_b):
    w2d = np.zeros((97, D), np.float32)
    for k in range(3):
        for c in range(TF):
            w2d[32 * k + c] = conv_w[:, c, k]
    w2d[96] = conv_b  # bias row; xsh row 96 is constant 1.0
    return w2d


def _pack_w1(w):
    """ffw1 [512, 2048] -> [128, 16m x (4kt x 128j)] block (m, kt) of 128x128."""
    return (w.reshape(4, 128, 16, 128).transpose(1, 2, 0, 3)
            .reshape(128, -1).copy())


def _pack_w2(w):
    """ffw2 [2048, 512] -> [128, 16m x 512] fp32 (block m = rows m*128..)."""
    return (w.reshape(16, 128, 512).transpose(1, 0, 2)
            .reshape(128, -1).copy())


def _hh(h):
    return slice(h * 512, (h + 1) * 512)


class K:
    def __init__(self):
        nc = bacc.Bacc()
        self.nc = nc
        p = nc.declare_dram_parameter
        self.d_xT = p("xT", [S * TF, N], F32, isOutput=False)
        self.d_w2d = p("w2d", [97, D], F32, isOutput=False)
        self.d_fm = p("fmats", [128, 13 * 128], F32, isOutput=False)
        self.d_im = p("imats", [128, 13 * 128], F32, isOutput=False)
        self.d_imr = p("imatsr", [128, 13 * 128], F32R, isOutput=False)
        self.d_pm = p("permm", [128, 128], F32, isOutput=False)
        self.d_jr = p("permj", [128, 128], F32, isOutput=False)
        self.d_idn = p("idn", [128, 128], F32, isOutput=False)
        self.d_e8 = p("e8", [HEADS, D], F32, isOutput=False)
        self.d_win = p("win", [L, 128, ND * D], F32R, isOutput=False)
        self.d_wout = p("wout", [L, 128, ND * D], F32R, isOutput=False)
        self.d_bout = p("boutr", [L, 1, D], F32, isOutput=False)
        self.d_al8 = p("alpha8", [L, HEADS, 1], F32, isOutput=False)
        self.d_ffw1p = p("ffw1p", [128, NM * D], F32R, isOutput=False)
        self.d_cpkp = p("cpkp", [128, 28], F32, isOutput=False)
        self.d_outwp = p("outwp", [128, ND * TF], F32, isOutput=False)
        self.d_lvw = p("lvwp2", [L, 128, 2 * ND * TF], F32, isOutput=False)
        self.d_lini = p("linip", [L, 128, 8], F32, isOutput=False)
        self.d_ffw2p = p("ffw2p", [128, NM * D], F32R, isOutput=False)
        self.d_gpost = p("gpostr", [1, D], F32, isOutput=False)
        self.d_bpost = p("bpostr", [1, D], F32, isOutput=False)
        self.d_bg = p("lvbg", [L, TF, 1], F32, isOutput=False)
        self.d_bp = p("lvbp", [L, TF, 1], F32, isOutput=False)
        self.d_alv = p("lvalpha", [L, 1, 1], F32, isOutput=False)
        self.d_damp = p("damp8", [HEADS, 1], F32, isOutput=False)
        self.d_outb = p("outbr", [1, TF], F32, isOutput=False)
        self.d_out = p("outT", [S * TF, HOR], F32, isOutput=True)
        self.zmid = nc.dram_tensor("zmid", [S, N, D], F32)
        self.xtmid = nc.dram_tensor("xtmid", [S, TF, N], F32)

    # psum bank helper: tag-based reuse of the 8 banks
    def bank(self, i, shape=(128, 512)):
        tl = self.psp.tile(list(shape), F32, tag=f"bk{i}", name=f"bk{i}")
        return tl

    def build(self):
        nc = self.nc
        with ExitStack() as ctx:
            self.tc = ctx.enter_context(tile.TileContext(nc))
            tc = self.tc
            top = ctx.enter_context(tc.tile_pool(name="top", bufs=1))

            idn = top.tile([128, 128], F32, name="idn")
            nc.sync.dma_start(idn[:], self.d_idn[:])
            ones = top.tile([128, 128], F32, name="ones")
            nc.vector.memset(ones[:], 1.0)
            fmt = top.tile([128, 13 * 128], F32, name="fmt")
            nc.sync.dma_start(fmt[:], self.d_fm[:])
            imrt = top.tile([128, 13 * 128], F32R, name="imrt")
            nc.sync.dma_start(imrt[:], self.d_imr[:])
            pmt = top.tile([128, 128], F32, name="pmt")
            nc.sync.dma_start(pmt[:], self.d_pm[:])
            jrt = top.tile([128, 128], F32, name="jrt")
            nc.sync.dma_start(jrt[:], self.d_jr[:])
            self.fmt, self.imrt, self.pmt, self.jrt = fmt, imrt, pmt, jrt
            e8 = top.tile([HEADS, D], F32, name="e8")
            nc.sync.dma_start(e8[:], self.d_e8[:])
            w2d = top.tile([97, D], F32, name="w2d")
            nc.sync.dma_start(w2d[:], self.d_w2d[:])
            # rows pack: p32 = gpost|bpost (2x512); p64 = outb[7]
            rows = top.tile([128, 512], F32, name="rows")
            nc.sync.dma_start(rows[32:33, 0:512], self.d_gpost[:])
            nc.sync.dma_start(rows[64:65, 0:512], self.d_bpost[:])
            nc.sync.dma_start(rows[0:1, 0:TF], self.d_outb[:])
            # col pack: gpre(4) | bpre(4) | ffb1(16) | ffb2(4)
            cpk = top.tile([128, 28], F32, name="cpk")
            nc.sync.dma_start(cpk[:], self.d_cpkp[:])
            outw = top.tile([128, ND * TF], F32, name="outw")
            nc.sync.dma_start(outw[:], self.d_outwp[:])
            eps = top.tile([128, 1], F32, name="eps")
            nc.vector.memset(eps[:], 1e-5)
            self.epst = eps
            agg = top.tile([128, S * ND * HOR], F32, name="agg")
            nc.vector.memset(agg[:], 0.0)
            csd = top.tile([128, ND * HOR], F32, name="csd")

            self.idn, self.ones, self.rows, self.cpk = idn, ones, rows, cpk
            self.e8t, self.w2dt_, self.aggt, self.csdt = e8, w2d, agg, csd
            self.outwt = outw

            # ff_post g/b replicated over 128 partitions (built once)
            gbt = top.tile([128, D], F32, name="gbt")
            bbt = top.tile([128, D], F32, name="bbt")
            self.gbt, self.bbt = gbt, bbt

            with tc.tile_pool(name="ini", bufs=1) as ini, \
                    tc.tile_pool(name="inips", bufs=1, space="PSUM") as inips:
                self.psp = inips
                self._damp_cs(ini, inips)

            for l in range(L):
                last = l == L - 1
                with tc.tile_pool(name=f"lay{l}", bufs=1) as layp, \
                        tc.tile_pool(name=f"wk{l}", bufs=1) as wk, \
                        tc.tile_pool(name=f"ps{l}", bufs=1, space="PSUM") as psp:
                    self.psp = psp
                    lay = self._layer_consts(l, layp)
                    for s in range(S):
                        self._sample(l, s, lay, wk)
                    if last:
                        for s in range(S):
                            self._output(s, wk)

        nc.compile()
        return nc

    # ---------- dampening cumsum -> csd [128, ND*HOR] ----------
    def _damp_cs(self, ini, inips):  # inips: any psum pool
        nc = self.nc
        ones = self.ones
        dcol = ini.tile([HEADS, 1], F32, name="dcol")
        nc.sync.dma_start(dcol[:], self.d_damp[:])
        df = ini.tile([HEADS, 1], F32, name="dfsig")
        nc.scalar.activation(df[:], dcol[:], AF.Sigmoid)
        dfb = ini.tile([HEADS, HOR], F32, name="dfb")
        nc.scalar.activation(dfb[:], ones[0:HEADS, 0:HOR], AF.Identity,
                             scale=df[:, 0:1])
        zer = ini.tile([HEADS, HOR], F32, name="zer8")
        nc.vector.memset(zer[:], 0.0)
        dfp = ini.tile([HEADS, HOR], F32, name="dfp")
        nc.vector.tensor_tensor_scan(dfp[:], dfb[:], zer[:], 1.0,
                                     op0=ALU.mult, op1=ALU.add)
        cs8 = ini.tile([HEADS, HOR], F32, name="cs8")
        nc.vector.tensor_tensor_scan(cs8[:], ones[0:HEADS, 0:HOR], dfp[:], 0.0,
                                     op0=ALU.mult, op1=ALU.add)
        for dt in range(ND):
            pini = self.bank(6)
            nc.tensor.matmul(pini[:, 0:HOR], self.e8t[:, _sl(dt)], cs8[:],
                             start=True, stop=True)
            nc.scalar.copy(self.csdt[:, dt * HOR:(dt + 1) * HOR], pini[:, 0:HOR])
        # replicate ff_post g/b rows across partitions
        pgb = self.bank(7)
        nc.tensor.matmul(pgb[:], ones[32:33, 0:128], self.rows[32:33, 0:512],
                         start=True, stop=True)
        nc.scalar.copy(self.gbt[:], pgb[:])
        pbb = self.bank(6)
        nc.tensor.matmul(pbb[:], ones[64:65, 0:128], self.rows[64:65, 0:512],
                         start=True, stop=True)
        nc.scalar.copy(self.bbt[:], pbb[:])

    # ---------- per-layer constants ----------
    def _layer_consts(self, l, layp):
        nc = self.nc
        ones = self.ones
        last = l == L - 1
        lay = {"l": l, "last": last}

        win1 = layp.tile([128, ND * D], F32R, name="win1")
        nc.sync.dma_start(win1[:], self.d_win[l, :, :])
        wout1 = layp.tile([128, ND * D], F32R, name="wout1")
        nc.sync.dma_start(wout1[:], self.d_wout[l, :, :])
        win = [win1[:, kt * D:(kt + 1) * D] for kt in range(ND)]
        wout = [wout1[:, kt * D:(kt + 1) * D] for kt in range(ND)]

        # lrows: p0 = bout[512]; level biases as [TF,1] columns for ACT bias
        lrows = layp.tile([128, 512], F32, name="lrows")
        nc.sync.dma_start(lrows[0:1, 0:D], self.d_bout[l, :, :])
        bgcol = layp.tile([TF, 1], F32, name="bgcol")
        nc.sync.dma_start(bgcol[:], self.d_bg[l, :, :])
        bpcol = layp.tile([TF, 1], F32, name="bpcol")
        nc.sync.dma_start(bpcol[:], self.d_bp[l, :, :])
        # bout replicated across partitions for the Pool-engine bias add
        boutt = layp.tile([128, D], F32, name="boutt")
        pbo = self.psp.tile([128, D], F32, tag="bk2", name="pbo")
        nc.tensor.matmul(pbo[:], ones[0:1, 0:128], lrows[0:1, 0:D],
                         start=True, stop=True)
        nc.scalar.copy(boutt[:], pbo[:])


        # lcol pack [128, 16]: al(4) oma(4) init(4) bi(4); plus lv cols [7,1]
        lcol = layp.tile([128, 24], F32, name="lcol")
        al8 = layp.tile([HEADS, 1], F32, tag="al8t", name="al8")
        nc.sync.dma_start(al8[:], self.d_al8[l, :, :])
        al8s = layp.tile([HEADS, 1], F32, tag="al8s", name="al8s")
        nc.scalar.activation(al8s[:], al8[:], AF.Sigmoid)
        for dt in range(ND):
            pal = self.psp.tile([128, 1], F32, tag="bk0", name="pal")
            nc.tensor.matmul(pal[:], self.e8t[:, _sl(dt)], al8s[:],
                             start=True, stop=True)
            nc.scalar.copy(lcol[:, dt:dt + 1], pal[:])
        libi = layp.tile([128, 8], F32, tag="libi", name="libi")
        nc.sync.dma_start(libi[:], self.d_lini[l, :, :])
        nc.vector.tensor_copy(lcol[:, 8:12], libi[:, 0:4])
        for dt in range(ND):
            nc.vector.tensor_scalar(lcol[:, 4 + dt:5 + dt], lcol[:, dt:dt + 1],
                                    -1.0, 1.0, op0=ALU.mult, op1=ALU.add)
        nc.vector.tensor_sub(lcol[:, 12:16], libi[:, 4:8], lcol[:, 8:12])
        # level alpha
        alv = layp.tile([1, 1], F32, tag="alvt", name="alv")
        nc.sync.dma_start(alv[:], self.d_alv[l, :, :])
        alvs = layp.tile([1, 1], F32, tag="alvst", name="alvs")
        nc.scalar.activation(alvs[:], alv[:], AF.Sigmoid)
        pv = self.psp.tile([TF, 1], F32, tag="bk1", name="palv")
        nc.tensor.matmul(pv[:], ones[0:1, 0:TF], alvs[:], start=True, stop=True)
        nc.scalar.copy(lcol[0:TF, 16:17], pv[:])
        nc.vector.tensor_scalar(lcol[0:TF, 17:18], lcol[0:TF, 16:17], -1.0, 1.0,
                                op0=ALU.mult, op1=ALU.add)

        # level weights [128, TF] x4 packed [128, 2*ND*TF], as fp32r
        lwf = layp.tile([128, 2 * ND * TF], F32, tag="lwf", name="lwf")
        nc.sync.dma_start(lwf[:], self.d_lvw[l, :, :])
        lw = layp.tile([128, 2 * ND * TF], F32R, name="lw")
        nc.vector.tensor_copy(lw[:], lwf[:])

        lay.update(win=win, wout=wout, lrows=lrows, lcol=lcol, lw=lw,
                   bgcol=bgcol, bpcol=bpcol, boutt=boutt)
        return lay

    # ---------- one sample through one layer ----------
    def _sample(self, l, s, lay, wk):
        nc = self.nc
        ones, idn = self.ones, self.idn
        last = lay["last"]
        agg = self.aggt

        def aggsl(dt):
            return self.aggt[:, (s * ND + dt) * HOR:(s * ND + dt + 1) * HOR]

        # --- z input: conv (l0) or reload (l1)
        z = [wk.tile([128, D], F32, tag=f"B1_{tt}", name=f"z{tt}")
             for tt in range(NT)]
        if l == 0:
            xsh = wk.tile([97, N], F32, tag="xd", name="xsh")
            xts = wk.tile([TF, N], F32, tag="xts", name="xts")
            nc.sync.dma_start(xts[:], self.d_xT[s * TF:(s + 1) * TF, :])
            nc.vector.memset(xsh[:], 0.0)
            nc.vector.tensor_copy(xsh[0:TF, 1:N], xts[:, 0:N - 1])
            nc.vector.tensor_copy(xsh[32:32 + TF, 0:N], xts[:, 0:N])
            nc.vector.tensor_copy(xsh[64:64 + TF, 0:N - 1], xts[:, 1:N])
            nc.vector.memset(xsh[96:97, :], 1.0)  # bias row (w2d row 96)
            for tt in range(NT):
                pz = self.bank(6 + tt % 2)
                nc.tensor.matmul(pz[:], xsh[:, _sl(tt)], self.w2dt_[:],
                                 start=True, stop=True)
                nc.scalar.copy(z[tt][:], pz[:])
        else:
            for tt in range(NT):
                nc.sync.dma_start(z[tt][:], self.zmid[s, _sl(tt), :])

        # ===== CT (8x128) rfft: stage 1 (DVE/Pool) -> G (B3), partials (B4)
        va, po = nc.vector, nc.gpsimd
        prt = [wk.tile([128, D], F32, tag=f"B4_{i}", name=f"prt{i}")
               for i in range(NT)]
        for u in range(4):
            va.tensor_add(prt[2 * u][:], z[u][:], z[u + 4][:])
            po.tensor_sub(prt[2 * u + 1][:], z[u][:], z[u + 4][:])
        a04, s04, a15, s15, a26, s26, a37, s37 = [p_[:] for p_ in prt]
        # G order: G0 G4 G1r G1i G2r G2i G3r G3i
        G = [wk.tile([128, D], F32, tag=f"B3_{i}", name=f"G{i}")
             for i in range(NT)]
        c0 = wk.tile([128, D], F32, tag="ct0", name="c0")
        c1t = wk.tile([128, D], F32, tag="ct1", name="c1t")
        va.tensor_add(c0[:], a04, a26)
        po.tensor_add(c1t[:], a15, a37)
        va.tensor_add(G[0][:], c0[:], c1t[:])
        va.tensor_sub(G[1][:], c0[:], c1t[:])
        po.tensor_sub(G[4][:], a04, a26)
        po.tensor_sub(G[5][:], a37, a15)
        qt = wk.tile([128, D], F32, tag="ct0", name="qt")
        va.tensor_sub(qt[:], s15, s37)
        pt_ = wk.tile([128, D], F32, tag="ct1", name="pt_")
        po.tensor_add(pt_[:], s15, s37)
        C1C = 0.7071067811865476
        va.scalar_tensor_tensor(G[2][:], qt[:], C1C, s04,
                                op0=ALU.mult, op1=ALU.add)
        va.scalar_tensor_tensor(G[3][:], pt_[:], -C1C, s26,
                                op0=ALU.mult, op1=ALU.subtract)
        va.scalar_tensor_tensor(G[6][:], qt[:], -C1C, s04,
                                op0=ALU.mult, op1=ALU.add)
        va.scalar_tensor_tensor(G[7][:], pt_[:], -C1C, s26,
                                op0=ALU.mult, op1=ALU.add)

        # ===== stage 2: X[k1] = [m,c] re|im (A2/X4); amp -> ampT [c, 640] (A1)
        fm = self.fmt
        xdt = F32R
        Xs = [wk.tile([128, 1024], xdt,
                      tag=(f"A2_{k1}" if k1 < 4 else "X4"), name=f"X{k1}")
              for k1 in range(5)]
        ampT = [wk.tile([128, 1024], F32, tag=f"A1_{ct}", name=f"ampT{ct}")
                for ct in range(ND)]
        FB = [0, 2, 5, 8, 11]

        def fmc(j):
            return fm[:, j * 128:(j + 1) * 128]

        for k1 in (0, 4, 2, 1, 3):
            bre = self.bank((2 * k1) % 6)
            bim = self.bank((2 * k1) % 6 + 1)
            b = FB[k1]
            if k1 in (0, 4):
                g = G[0] if k1 == 0 else G[1]
                nc.tensor.matmul(bre[:], fmc(b), g[:], start=True, stop=True)
                nc.tensor.matmul(bim[:], fmc(b + 1), g[:], start=True, stop=True)
            else:
                gr, gi = G[2 * k1], G[2 * k1 + 1]
                nc.tensor.matmul(bre[:], fmc(b), gr[:], start=True, stop=False)
                nc.tensor.matmul(bre[:], fmc(b + 1), gi[:], start=False, stop=True)
                nc.tensor.matmul(bim[:], fmc(b), gi[:], start=True, stop=False)
                nc.tensor.matmul(bim[:], fmc(b + 2), gr[:], start=False, stop=True)
            sq0 = wk.tile([128, D], F32, tag="sq0", name="sq0")
            nc.scalar.activation(sq0[:], bre[:], AF.Square)
            sq1 = wk.tile([128, D], F32, tag="sq1", name="sq1")
            nc.scalar.activation(sq1[:], bim[:], AF.Square)
            nc.scalar.copy(Xs[k1][:, 0:512], bre[:])
            nc.scalar.copy(Xs[k1][:, 512:1024], bim[:])
            amp = wk.tile([128, D], F32, tag=f"amp{k1 % 2}", name=f"amp{k1}")
            va.tensor_add(amp[:], sq0[:], sq1[:])
            pT = self.bank(6 + k1 % 2)
            for ct in range(ND):
                nc.tensor.transpose(pT[:, _sl(ct)], amp[:, _sl(ct)], idn[:])
            for ct in range(ND):
                nc.scalar.copy(ampT[ct][:, k1 * 128:(k1 + 1) * 128],
                               pT[:, _sl(ct)])
            if k1 in (0, 4):
                # mirror-duplicate zones via permutation matmul (exact copies)
                pM = self.bank(7 - k1 % 2)
                pmat = self.pmt if k1 == 0 else self.jrt
                for ct in range(ND):
                    nc.tensor.matmul(pM[:, _sl(ct)], amp[:, _sl(ct)], pmat[:],
                                     start=True, stop=True)
                off = 65 if k1 == 0 else 512 + 64
                lo = 65 if k1 == 0 else 64
                for ct in range(ND):
                    nc.scalar.copy(ampT[ct][:, off:(k1 * 128 + 128)],
                                   pM[:, ct * 128 + lo:(ct + 1) * 128])

        # ===== ranking: canon top-4 over [0:65] U [128:576]; in-place mask
        for ct in range(ND):
            t16 = wk.tile([128, 16], F32, tag=f"t16_{ct}", name="t16")
            va.max(t16[:, 0:8], ampT[ct][:, 0:65])
            va.max(t16[:, 8:16], ampT[ct][:, 128:576])
            top8 = wk.tile([128, 8], F32, tag=f"top8_{ct}", name="top8")
            va.max(top8[:], t16[:])
            va.tensor_scalar(ampT[ct][:, 0:640], ampT[ct][:, 0:640],
                             top8[:, 3:4], 0.0, op0=ALU.is_ge, op1=ALU.add)

        # ===== mask transpose per k1; apply to X
        for k1 in range(5):
            pM = self.bank(6 + k1 % 2)
            for ct in range(ND):
                nc.tensor.transpose(pM[:, _sl(ct)],
                                    ampT[ct][:, k1 * 128:(k1 + 1) * 128], idn[:])
            va.tensor_mul(Xs[k1][:, 0:512], Xs[k1][:, 0:512], pM[:])
            va.tensor_mul(Xs[k1][:, 512:1024], Xs[k1][:, 512:1024], pM[:])

        # ===== inverse: 16 matmuls -> 8 banks -> W (B4)
        imt = self.imrt
        W = [wk.tile([128, D], F32, tag=f"B4_{i}", name=f"W{i}")
             for i in range(NT)]

        def imc(j):
            return imt[:, j * 128:(j + 1) * 128]

        pb = [self.bank(i) for i in range(8)]
        nc.tensor.matmul(pb[0][:], imc(0), Xs[0][:, 0:512],
                         start=True, stop=False)
        nc.tensor.matmul(pb[0][:], imc(1), Xs[0][:, 512:1024],
                         start=False, stop=True)
        nc.tensor.matmul(pb[1][:], imc(11), Xs[4][:, 0:512],
                         start=True, stop=False)
        nc.tensor.matmul(pb[1][:], imc(12), Xs[4][:, 512:1024],
                         start=False, stop=True)
        for k1 in (1, 2, 3):
            b = FB[k1]
            br, bi = pb[2 * k1], pb[2 * k1 + 1]
            nc.tensor.matmul(br[:], imc(b), Xs[k1][:, 0:512],
                             start=True, stop=False)
            nc.tensor.matmul(br[:], imc(b + 2), Xs[k1][:, 512:1024],
                             start=False, stop=True)
            nc.tensor.matmul(bi[:], imc(b + 1), Xs[k1][:, 0:512],
                             start=True, stop=False)
            nc.tensor.matmul(bi[:], imc(b), Xs[k1][:, 512:1024],
                             start=False, stop=True)
        for i in range(8):
            nc.scalar.copy(W[i][:], pb[i][:])

        # ===== combine -> lp (B3); W order: W0 P4 Z1r Z1i Z2r Z2i Z3r Z3i
        lp = [wk.tile([128, D], F32, tag=f"B3_{tt}", name=f"lp{tt}")
              for tt in range(NT)]
        A_ = wk.tile([128, D], F32, tag="ct0", name="A_")
        Bm = wk.tile([128, D], F32, tag="ct1", name="Bm")
        va.tensor_add(A_[:], W[0][:], W[1][:])
        po.tensor_sub(Bm[:], W[0][:], W[1][:])
        R13p = wk.tile([128, D], F32, tag="sq0", name="R13p")
        va.tensor_add(R13p[:], W[2][:], W[6][:])
        R13m = wk.tile([128, D], F32, tag="sq1", name="R13m")
        po.tensor_sub(R13m[:], W[2][:], W[6][:])
        I13p = wk.tile([128, D], F32, tag="amp0", name="I13p")
        va.tensor_add(I13p[:], W[3][:], W[7][:])
        I13m = wk.tile([128, D], F32, tag="amp1", name="I13m")
        po.tensor_sub(I13m[:], W[3][:], W[7][:])
        va.tensor_add(W[0][:], A_[:], W[4][:])       # E0
        po.tensor_sub(W[1][:], Bm[:], W[5][:])       # E1
        va.tensor_sub(A_[:], A_[:], W[4][:])         # E2
        po.tensor_add(Bm[:], Bm[:], W[5][:])         # E3
        va.tensor_sub(W[2][:], R13m[:], I13p[:])     # q1
        po.tensor_add(W[3][:], R13m[:], I13p[:])     # q3
        va.tensor_add(lp[0][:], W[0][:], R13p[:])
        po.tensor_sub(lp[4][:], W[0][:], R13p[:])
        va.scalar_tensor_tensor(lp[1][:], W[2][:], C1C, W[1][:],
                                op0=ALU.mult, op1=ALU.add)
        va.scalar_tensor_tensor(lp[5][:], W[2][:], -C1C, W[1][:],
                                op0=ALU.mult, op1=ALU.add)
        va.tensor_sub(lp[2][:], A_[:], I13m[:])
        po.tensor_add(lp[6][:], A_[:], I13m[:])
        va.scalar_tensor_tensor(lp[3][:], W[3][:], -C1C, Bm[:],
                                op0=ALU.mult, op1=ALU.add)
        va.scalar_tensor_tensor(lp[7][:], W[3][:], C1C, Bm[:],
                                op0=ALU.mult, op1=ALU.add)
        z2 = [wk.tile([128, D], F32, tag=f"B4_{tt}", name=f"z2_{tt}")
              for tt in range(NT)]
        for tt in range(NT):
            eng = va if tt % 2 == 0 else po
            eng.tensor_sub(z2[tt][:], z[tt][:], lp[tt][:])

        # --- lpT [ND][128, N] (tag A2) + extrap + perT; then free
        lpT = [wk.tile([128, N], F32R, tag=f"A2_{dt}", name=f"lpT{dt}")
               for dt in range(ND)]
        for dt in range(ND):
            for h in range(2):
                pT = self.bank(dt % 2)
                for q in range(4):
                    nc.tensor.transpose(pT[:, _sl(q)], lp[h * 4 + q][:, _sl(dt)],
                                        idn[:])
                nc.scalar.copy(lpT[dt][:, _hh(h)], pT[:])
            nc.vector.tensor_add(aggsl(dt), aggsl(dt), lpT[dt][:, 0:HOR])
        perT = wk.tile([TF, N], F32, tag="perT", name="perT")
        for h in range(2):
            pp = self.bank(2)
            for kt in range(ND):
                nc.tensor.matmul(pp[0:TF, :], lay["lw"][:, (ND + kt) * TF:(ND + kt + 1) * TF],
                                 lpT[kt][:, _hh(h)], start=(kt == 0),
                                 stop=(kt == ND - 1))
            nc.scalar.activation(perT[:, _hh(h)], pp[0:TF, :], AF.Identity,
                                 bias=lay["bpcol"][:, 0:1])

        # --- z2T (tag A2 reuse after lpT dead)
        z2T = [wk.tile([128, N], F32R, tag=f"A2_{dt}", name=f"z2T{dt}")
               for dt in range(ND)]
        for dt in range(ND):
            for h in range(2):
                pT = self.bank(dt % 2)
                for q in range(4):
                    nc.tensor.transpose(pT[:, _sl(q)], z2[h * 4 + q][:, _sl(dt)],
                                        idn[:])
                nc.vector.tensor_copy(z2T[dt][:, _hh(h)], pT[:])

        # --- win GEMM -> xinT (tag A1 reuse: filt dead)
        xinT = [wk.tile([128, N], F32, tag=f"A1_{dt}", name=f"xinT{dt}")
                for dt in range(ND)]
        for dt in range(ND):
            for h in range(2):
                px = self.bank(4 + dt % 2)
                for kt in range(ND):
                    nc.tensor.matmul(px[:], lay["win"][kt][:, _sl(dt)],
                                     z2T[kt][:, _hh(h)],
                                     start=(kt == 0), stop=(kt == ND - 1))
                nc.scalar.copy(xinT[dt][:, _hh(h)], px[:])

        # --- xd -> scan -> sT (tag A2 reuse: z2T dead)
        sT = [wk.tile([128, N], F32R, tag=f"A2_{dt}", name=f"sT{dt}")
              for dt in range(ND)]
        lc = lay["lcol"]
        for dt in range(ND):
            xd = wk.tile([128, N], F32, tag="xd", name="xd")
            nc.vector.tensor_sub(xd[:, 1:N], xinT[dt][:, 1:N], xinT[dt][:, 0:N - 1])
            nc.vector.tensor_scalar_add(xd[:, 0:1], xinT[dt][:, 0:1],
                                        lc[:, 12 + dt:13 + dt])
            nc.vector.tensor_scalar_mul(xd[:], xd[:], lc[:, dt:dt + 1])
            if OMA_BCAST:
                omab_ap = lc[:, 4 + dt:5 + dt].broadcast_to([128, N])
            else:
                omab = wk.tile([128, N], F32, tag="omab", name="omab")
                nc.vector.memset(omab[:], 1.0)
                nc.vector.tensor_scalar_mul(omab[:], omab[:], lc[:, 4 + dt:5 + dt])
                omab_ap = omab[:]
            nc.vector.tensor_tensor_scan(sT[dt][:], omab_ap, xd[:],
                                         lc[:, 8 + dt:9 + dt],
                                         op0=ALU.mult, op1=ALU.add)

        # --- wout GEMM -> lg [t,d] (tag B2 reuse: filtT dead) (+ z3 if l0)
        lg = [wk.tile([128, D], F32, tag=f"B2_{tt}", name=f"lg{tt}")
              for tt in range(NT)]
        for tt in range(NT):
            pg = self.bank(tt % 2)
            for kt in range(ND):
                nc.tensor.matmul(pg[:], sT[kt][:, _sl(tt)], lay["wout"][kt],
                                 start=(kt == 0), stop=(kt == ND - 1))
            nc.vector.tensor_add(lg[tt][:], pg[:], lay["boutt"][:])
            if not last:
                # z3 overwrites z (tag B1): z dead after z2
                nc.vector.tensor_sub(z[tt][:], z2[tt][:], lg[tt][:])
        z3 = z

        # --- lgT via transposes (tag A1 reuse: xinT dead)
        lgT = [wk.tile([128, N], F32R, tag=f"A1_{dt}", name=f"lgT{dt}")
               for dt in range(ND)]
        for dt in range(ND):
            for h in range(2):
                pT = self.bank(2 + dt % 2)
                for q in range(4):
                    nc.tensor.transpose(pT[:, _sl(q)], lg[h * 4 + q][:, _sl(dt)],
                                        idn[:])
                nc.scalar.copy(lgT[dt][:, _hh(h)], pT[:])
            # damp: agg += lg_last * csd
            nc.vector.scalar_tensor_tensor(
                aggsl(dt), self.csdt[:, dt * HOR:(dt + 1) * HOR],
                lgT[dt][:, N - 1:N], aggsl(dt), op0=ALU.mult, op1=ALU.add)

        # --- level: grT; scans update xtmid
        grT = wk.tile([TF, N], F32, tag="grT", name="grT")
        for h in range(2):
            pgr = self.bank(4)
            for kt in range(ND):
                nc.tensor.matmul(pgr[0:TF, :], lay["lw"][:, kt * TF:(kt + 1) * TF],
                                 lgT[kt][:, _hh(h)], start=(kt == 0),
                                 stop=(kt == ND - 1))
            nc.scalar.activation(grT[:, _hh(h)], pgr[0:TF, :], AF.Identity,
                                 bias=lay["bgcol"][:, 0:1])

        xts2 = wk.tile([TF, N], F32, tag="xts", name="xts2")
        if l == 0:
            nc.sync.dma_start(xts2[:], self.d_xT[s * TF:(s + 1) * TF, :])
        else:
            nc.sync.dma_start(xts2[:], self.xtmid[s, :, :])
        v = wk.tile([TF, N], F32, tag="lvv", name="lvv")
        nc.vector.tensor_sub(v[:], xts2[:], perT[:])
        nc.vector.tensor_scalar_mul(v[:], v[:], lc[0:TF, 16:17])
        if OMA_BCAST:
            omlv_ap = lc[0:TF, 17:18].broadcast_to([TF, N])
        else:
            omlv = wk.tile([TF, N], F32, tag="omlv", name="omlv")
            nc.vector.memset(omlv[:], 1.0)
            nc.vector.tensor_scalar_mul(omlv[:], omlv[:], lc[0:TF, 17:18])
            omlv_ap = omlv[:]
        pt = wk.tile([TF, N], F32, tag="lvp", name="lvp")
        nc.vector.tensor_tensor_scan(pt[:], omlv_ap, v[:], 0.0,
                                     op0=ALU.mult, op1=ALU.add)
        gt = wk.tile([TF, N], F32, tag="lvv", name="lvg")
        nc.vector.tensor_tensor_scan(gt[:], omlv_ap, grT[:], 0.0,
                                     op0=ALU.mult, op1=ALU.add)
        xnew = wk.tile([TF, N], F32, tag="grT", name="xnew")
        nc.vector.tensor_add(xnew[:], pt[:], gt[:])
        nc.sync.dma_start(self.xtmid[s, :, :], xnew[:])

        # --- FF (layer 0 only), then spill z4
        if not last:
            z4 = self._ff(s, z3, wk)
            for tt in range(NT):
                nc.sync.dma_start(self.zmid[s, _sl(tt), :], z4[tt][:])

    # ---------- LN stats ----------
    def _ln_stats(self, zset, wk, tagp):
        nc = self.nc
        st = wk.tile([128, 8 * NT], F32, tag=f"st{tagp}", name=f"st{tagp}")
        mu8 = st[:, 0:NT]
        s28 = st[:, NT:2 * NT]
        scr = wk.tile([128, D], F32, tag="lnscr", name="lnscr")
        for tt in range(NT):
            nc.vector.tensor_reduce(st[:, tt:tt + 1], zset[tt][:],
                                    mybir.AxisListType.X, op=ALU.add)
            nc.scalar.activation(scr[:], zset[tt][:], AF.Square,
                                 accum_out=st[:, NT + tt:NT + tt + 1])
        mun = st[:, 2 * NT:3 * NT]
        nc.vector.tensor_scalar_mul(mun, mu8, 1.0 / D)
        ex2 = st[:, 3 * NT:4 * NT]
        nc.vector.tensor_scalar_mul(ex2, s28, 1.0 / D)
        musq = st[:, 4 * NT:5 * NT]
        nc.scalar.activation(musq, mun, AF.Square)
        var = st[:, 5 * NT:6 * NT]
        nc.vector.tensor_sub(var, ex2, musq)
        sd = st[:, 6 * NT:7 * NT]
        nc.scalar.activation(sd, var, AF.Sqrt, bias=self.epst[:, 0:1])
        rs = st[:, 7 * NT:8 * NT]
        nc.vector.reciprocal(rs, sd)
        nmurs = st[:, 4 * NT:5 * NT]  # overwrite musq slot
        nc.vector.tensor_mul(nmurs, mun, rs)
        nc.vector.tensor_scalar_mul(nmurs, nmurs, -1.0)
        return rs, nmurs

    # ---------- FF block ----------
    def _ff(self, s, z3, wk):
        nc = self.nc
        ones, idn = self.ones, self.idn
        rows, cpk = self.rows, self.cpk
        rs, nmurs = self._ln_stats(z3, wk, "pre")
        # h = (z3-mu)*rs, overwrite z3 tiles in place via scratch
        h_ = [wk.tile([128, D], F32, tag=f"B2_{tt}", name=f"h{tt}")
              for tt in range(NT)]
        for tt in range(NT):
            nc.scalar.activation(h_[tt][:], z3[tt][:], AF.Identity,
                                 scale=rs[:, tt:tt + 1], bias=nmurs[:, tt:tt + 1])
        hT = [wk.tile([128, N], F32, tag=f"A2_{dt}", name=f"hT{dt}")
              for dt in range(ND)]
        for dt in range(ND):
            for h in range(2):
                pT = self.bank(dt % 2)
                for q in range(4):
                    nc.tensor.transpose(pT[:, _sl(q)], h_[h * 4 + q][:, _sl(dt)],
                                        idn[:])
                nc.scalar.copy(hT[dt][:, _hh(h)], pT[:])
        znT = [wk.tile([128, N], F32R, tag=f"A1_{dt}", name=f"znT{dt}")
               for dt in range(ND)]
        for dt in range(ND):
            nc.scalar.activation(znT[dt][:], hT[dt][:], AF.Identity,
                                 scale=cpk[:, dt:dt + 1], bias=cpk[:, 4 + dt:5 + dt])

        yT = [wk.tile([128, N], F32, tag=f"A2_{dt}", name=f"yT{dt}")
              for dt in range(ND)]
        for h in range(2):
            pzf = [self.bank(dt) for dt in range(ND)]
            for m in range(NM):
                w1m = wk.tile([128, D], F32R, tag=f"w1m{m % 2}", name="w1m")
                nc.sync.dma_start(w1m[:], self.d_ffw1p[:, m * D:(m + 1) * D])
                w2m = wk.tile([128, D], F32R, tag=f"w2m{m % 2}", name="w2m")
                nc.sync.dma_start(w2m[:], self.d_ffw2p[:, m * D:(m + 1) * D])
                ph = self.bank(4 + m % 2)
                for kt in range(ND):
                    nc.tensor.matmul(
                        ph[:], w1m[:, kt * 128:(kt + 1) * 128],
                        znT[kt][:, _hh(h)],
                        start=(kt == 0), stop=(kt == ND - 1))
                sig = wk.tile([128, 512], F32R, tag=f"sig{m % 2}", name="sig")
                nc.scalar.activation(sig[:], ph[:], AF.Sigmoid,
                                     bias=cpk[:, 8 + m:9 + m])
                for dt in range(ND):
                    nc.tensor.matmul(pzf[dt][:], w2m[:, dt * 128:(dt + 1) * 128],
                                     sig[:], start=(m == 0), stop=(m == NM - 1))
            for dt in range(ND):
                nc.vector.scalar_tensor_tensor(yT[dt][:, _hh(h)], pzf[dt][:],
                                               cpk[:, 24 + dt:25 + dt],
                                               znT[dt][:, _hh(h)].bitcast(F32),
                                               op0=ALU.add, op1=ALU.add)

        y = [wk.tile([128, D], F32, tag=f"B4_{tt}", name=f"y{tt}")
             for tt in range(NT)]
        for tt in range(NT):
            pT = self.bank(2 + tt % 2)
            for dt in range(ND):
                nc.tensor.transpose(pT[:, _sl(dt)], yT[dt][:, _sl(tt)], idn[:])
            nc.scalar.copy(y[tt][:], pT[:])

        rs2, nmurs2 = self._ln_stats(y, wk, "post")
        z4 = [wk.tile([128, D], F32, tag=f"B1_{tt}", name=f"z4_{tt}")
              for tt in range(NT)]
        scr2 = wk.tile([128, D], F32, tag="lnscr", name="scr2")
        for tt in range(NT):
            nc.scalar.activation(scr2[:], y[tt][:], AF.Identity,
                                 scale=rs2[:, tt:tt + 1], bias=nmurs2[:, tt:tt + 1])
            nc.vector.tensor_mul(z4[tt][:], scr2[:], self.gbt[:])
            nc.vector.tensor_add(z4[tt][:], z4[tt][:], self.bbt[:])
        return z4

    # ---------- output head ----------
    def _output(self, s, wk):
        nc = self.nc
        ones = self.ones
        po = self.bank(7)
        for kt in range(ND):
            nc.tensor.matmul(po[0:TF, 0:HOR], self.outwt[:, kt * TF:(kt + 1) * TF],
                             self.aggt[:, (s * ND + kt) * HOR:(s * ND + kt + 1) * HOR],
                             start=(kt == 0), stop=False)
        nc.tensor.matmul(po[0:TF, 0:HOR], self.rows[0:1, 0:TF],
                         ones[0:1, 0:HOR], start=False, stop=True)
        xfin = wk.tile([TF, N], F32, tag="xts", name="xfin")
        nc.sync.dma_start(xfin[:], self.xtmid[s, :, :])
        oT = wk.tile([TF, HOR], F32, tag="oT", name="oT")
        nc.vector.tensor_scalar_add(oT[:], po[0:TF, 0:HOR], xfin[:, N - 1:N])
        nc.sync.dma_start(self.d_out[s * TF:(s + 1) * TF, :], oT[:])


def _get_nc():
    if "nc" not in _CACHE:
        _CACHE["nc"] = K().build()
    return _CACHE["nc"]


def _common_maps(inputs, w2d, cts, e8):
    ffw1 = np.asarray(inputs["ff_w1"], np.float32)
    ffw2 = np.asarray(inputs["ff_w2"], np.float32)
    fm, im, pm, jr = cts
    return dict(
        w2d=w2d,
        fmats=fm, imats=im, imatsr=im, permm=pm, permj=jr,
        idn=np.eye(128, dtype=np.float32),
        e8=e8,
        win=np.asarray(inputs["mhesa_win"], np.float32)
        .reshape(L, 4, 128, D).transpose(0, 2, 1, 3).reshape(L, 128, -1).copy(),
        wout=np.asarray(inputs["mhesa_wout"], np.float32)
        .reshape(L, 4, 128, D).transpose(0, 2, 1, 3).reshape(L, 128, -1).copy(),
        boutr=np.asarray(inputs["mhesa_bout"], np.float32).reshape(L, 1, D),
        alpha8=np.asarray(inputs["mhesa_alpha"], np.float32).reshape(L, HEADS, 1),
        ffw1p=_pack_w1(ffw1),
        ffw2p=_pack_w2(ffw2),
        cpkp=np.concatenate([
            np.asarray(inputs["ff_pre_g"], np.float32).reshape(4, 128).T,
            np.asarray(inputs["ff_pre_b"], np.float32).reshape(4, 128).T,
            np.asarray(inputs["ff_b1"], np.float32).reshape(16, 128).T,
            np.asarray(inputs["ff_b2"], np.float32).reshape(4, 128).T,
        ], axis=1),
        outwp=np.asarray(inputs["out_w"], np.float32)
        .reshape(4, 128, TF).transpose(1, 0, 2).reshape(128, -1).copy(),
        lvwp2=np.concatenate([
            np.asarray(inputs["level_wg"], np.float32)
            .reshape(L, 4, 128, TF).transpose(0, 2, 1, 3).reshape(L, 128, -1),
            np.asarray(inputs["level_wp"], np.float32)
            .reshape(L, 4, 128, TF).transpose(0, 2, 1, 3).reshape(L, 128, -1),
        ], axis=2),
        linip=np.concatenate([
            np.asarray(inputs["mhesa_init"], np.float32)
            .reshape(L, 4, 128).transpose(0, 2, 1),
            np.asarray(inputs["mhesa_bin"], np.float32)
            .reshape(L, 4, 128).transpose(0, 2, 1),
        ], axis=2),
        gpostr=np.asarray(inputs["ff_post_g"], np.float32).reshape(1, D),
        bpostr=np.asarray(inputs["ff_post_b"], np.float32).reshape(1, D),
        lvbg=np.asarray(inputs["level_bg"], np.float32).reshape(L, TF, 1),
        lvbp=np.asarray(inputs["level_bp"], np.float32).reshape(L, TF, 1),
        lvalpha=np.asarray(inputs["level_alpha"], np.float32).reshape(L, 1, 1),
        damp8=np.asarray(inputs["dampen_factor"], np.float32).reshape(HEADS, 1),
        outw=np.asarray(inputs["out_w"], np.float32),
        outbr=np.asarray(inputs["out_b"], np.float32).reshape(1, TF),
    )


def _kernel_impl(inputs, runner):
    x = np.asarray(inputs["x"], np.float32)
    assert (x.shape[0], x.shape[1], x.shape[2]) == (32, N, TF)
    assert int(inputs["forecast_horizon"]) == HOR
    cts = _ct_consts()
    conv_w = np.asarray(inputs["conv_w"], np.float32)
    w2d = _build_w2d(conv_w, np.asarray(inputs["conv_b"], np.float32))
    e8 = np.repeat(np.eye(HEADS, dtype=np.float32), DH, axis=1)
    nc = _get_nc()
    common = _common_maps(inputs, w2d, cts, e8)
    in_maps = []
    for c in range(NCORES):
        xs = x[c * S:(c + 1) * S]
        xT = xs.transpose(0, 2, 1).reshape(S * TF, N).copy()
        in_maps.append(dict(common, xT=xT))
    res = runner(nc, in_maps)
    out = np.zeros((x.shape[0], HOR, TF), np.float32)
    for c in range(NCORES):
        oT = res.results[c]["outT"].reshape(S, TF, HOR)
        out[c * S:(c + 1) * S] = oT.transpose(0, 2, 1)
    return out, res


def kernel(**inputs):
    out, _ = _kernel_impl(
        inputs,
        lambda nc, im: run_bass_kernel_spmd(nc, im, list(range(NCORES))))
    return out


def kernel_traced(**inputs):
    """Like kernel() but with NTFF profiling; returns (out, BassKernelResults)."""
    return _kernel_impl(
        inputs,
        lambda nc, im: run_bass_kernel_spmd(nc, im, list(range(NCORES)),
                                            trace=True))



# revision 55
# speedup vs baseline: 1.0269x; 1.0269x over previous
"""ETSFormer forward pass on 8 Trainium2 NeuronCores (Bass/Tile).

Data-parallel over batch: 32 samples -> 8 cores x 4 samples, weights
replicated, no collectives. The reference's FFT machinery is computed
exactly without dense FFT matmuls:
  - freq_attention: Cooley-Tukey 1024 = 8x128 factorization. Inner 8-point
    stage = DVE/Pool linear combos of the eight [128,512] z tiles; outer
    128-point stage = single-pass fp32 matmuls contracting over partitions,
    with the twiddles e^{2 pi i v(8m+k1)/1024} folded into 13 per-k1
    stationary [128,128] matrices (fmats/imats). Frequencies are indexed
    f = k1 + 8m, k1 = 0..4 (k1 > 4 via conjugate symmetry); top-4 ranking
    happens over a transposed amp layout [c, (k1,m)] with mirror-duplicate
    zones rebuilt by permutation matmuls so dedup matches the reference's
    513-frequency ranking exactly; the keep-mask applies in [m, c] layout
    and the inverse CT reverses the factorization (combine on DVE/Pool).
  - mhesa / level exponential smoothing: first-order EMA -> hardware
    prefix scan (tensor_tensor_scan); fourier_extrapolate: exact slice.

Precision: the top-4 ranking is extremely sensitive (2e-4 relative amp
noise can flip ranks -> 1e-2-class output error), so the CT forward path
(conv, stage-1 combos, stage-2 matmuls, amp) is exact fp32 in BOTH layers.
fp32r (1 cyc/row vs fp32's 4, ~2e-4 relative truncation) is used for the
mhesa win/wout, FF w1/w2, the inverse CT of both layers, level and output
paths; measured rel err 1.08e-2 vs the 2e-2 gate (deterministic for fixed
inputs). Host-side packing loads each constant group in one DMA (HWDGE
charges 625ns fixed per dma_start, so DMA count dominates transfer cost).
"""
import numpy as np
from contextlib import ExitStack

import concourse.bass as bass
import concourse.bacc as bacc
import concourse.tile as tile
from concourse import mybir
from concourse.bass_utils import run_bass_kernel_spmd

F32 = mybir.dt.float32
F32R = mybir.dt.float32r
BF16 = mybir.dt.bfloat16
AF = mybir.ActivationFunctionType
ALU = mybir.AluOpType

N = 1024
D = 512
TF = 7
HEADS = 8
DH = D // HEADS
L = 2
S = 4
NCORES = 8
HOR = 96
FD = 2048
NT = N // 128   # 8
ND = D // 128   # 4
NM = FD // 128  # 16

_CACHE = {}
OMA_BCAST = True


def _ct_consts():
    """Folded-twiddle CT-DFT matrices.
    fmats [128, 13*128]: [FC0, FS0n | FC1, FS1, FS1n | ... | FC4, FS4n],
      FCk1[v, m] = cos(2 pi v (8m+k1) / 1024), FS = sin, *n = negated.
    imats: inverse, [m, v]-layout, scaled by (1 or 2)/1024.
    pm/jr: permutation matrices for mirror-duplicate zones."""
    if "fm" not in _CACHE:
        v = np.arange(128)
        m = np.arange(128)
        fmats, imats = [], []
        for k1 in range(5):
            th = 2.0 * np.pi * np.outer(v, 8 * m + k1) / N
            FC, FS = np.cos(th), np.sin(th)
            sc = (1.0 if k1 in (0, 4) else 2.0) / N
            thi = 2.0 * np.pi * np.outer(8 * m + k1, v) / N
            IC, IS = np.cos(thi) * sc, np.sin(thi) * sc
            if k1 in (0, 4):
                fmats += [FC, -FS]
                imats += [IC, -IS]
            else:
                fmats += [FC, FS, -FS]
                imats += [IC, IS, -IS]
        _CACHE["fm"] = np.concatenate(fmats, axis=1).astype(np.float32)
        _CACHE["im"] = np.concatenate(imats, axis=1).astype(np.float32)
        pm = np.zeros((128, 128), np.float32)
        pm[(128 - np.arange(128)) % 128, np.arange(128)] = 1.0
        jr = np.zeros((128, 128), np.float32)
        jr[127 - np.arange(128), np.arange(128)] = 1.0
        _CACHE["pm"] = pm
        _CACHE["jr"] = jr
    return _CACHE["fm"], _CACHE["im"], _CACHE["pm"], _CACHE["jr"]


def _sl(i, w=128):
    return slice(i * w, (i + 1) * w)


def _build_w2d(conv_w, conv_b):
    w2d = np.zeros((97, D), np.float32)
    for k in range(3):
        for c in range(TF):
            w2d[32 * k + c] = conv_w[:, c, k]
    w2d[96] = conv_b  # bias row; xsh row 96 is constant 1.0
    return w2d


def _pack_w1(w):
    """ffw1 [512, 2048] -> [128, 16m x (4kt x 128j)] block (m, kt) of 128x128."""
    return (w.reshape(4, 128, 16, 128).transpose(1, 2, 0, 3)
            .reshape(128, -1).copy())


def _pack_w2(w):
    """ffw2 [2048, 512] -> [128, 16m x 512] fp32 (block m = rows m*128..)."""
    return (w.reshape(16, 128, 512).transpose(1, 0, 2)
            .reshape(128, -1).copy())


def _hh(h):
    return slice(h * 512, (h + 1) * 512)


def _al_d(inputs):
    """sigmoid(mhesa_alpha) expanded per model dim: [L, D]."""
    al = np.asarray(inputs["mhesa_alpha"], np.float32)
    al = 1.0 / (1.0 + np.exp(-al.astype(np.float64)))
    return np.repeat(al, DH, axis=1).astype(np.float32)


class K:
    def __init__(self):
        nc = bacc.Bacc()
        self.nc = nc
        p = nc.declare_dram_parameter
        self.d_xT = p("xT", [S * TF, N], F32, isOutput=False)
        self.d_w2d = p("w2d", [97, D], F32, isOutput=False)
        self.d_fm = p("fmats", [128, 13 * 128], F32, isOutput=False)
        self.d_im = p("imats", [128, 13 * 128], F32, isOutput=False)
        self.d_imr = p("imatsr", [128, 13 * 128], F32R, isOutput=False)
        self.d_pm = p("permm", [128, 128], F32, isOutput=False)
        self.d_jr = p("permj", [128, 128], F32, isOutput=False)
        self.d_idn = p("idn", [128, 128], F32, isOutput=False)
        self.d_e8 = p("e8", [HEADS, D], F32, isOutput=False)
        self.d_win = p("win", [L, 128, ND * D], F32R, isOutput=False)
        self.d_wout = p("wout", [L, 128, ND * D], F32R, isOutput=False)
        self.d_bout = p("boutr", [L, 1, D], F32, isOutput=False)
        self.d_al8 = p("alpha8", [L, HEADS, 1], F32, isOutput=False)
        self.d_ffw1p = p("ffw1p", [128, NM * D], F32R, isOutput=False)
        self.d_cpkp = p("cpkp", [128, 28], F32, isOutput=False)
        self.d_outwp = p("outwp", [128, ND * TF], F32, isOutput=False)
        self.d_lvw = p("lvwp2", [L, 128, 2 * ND * TF], F32, isOutput=False)
        self.d_lini = p("linip", [L, 128, 8], F32, isOutput=False)
        self.d_ffw2p = p("ffw2p", [128, NM * D], F32R, isOutput=False)
        self.d_gpost = p("gpostr", [1, D], F32, isOutput=False)
        self.d_bpost = p("bpostr", [1, D], F32, isOutput=False)
        self.d_bg = p("lvbg", [L, TF, 1], F32, isOutput=False)
        self.d_bp = p("lvbp", [L, TF, 1], F32, isOutput=False)
        self.d_alv = p("lvalpha", [L, 1, 1], F32, isOutput=False)
        self.d_damp = p("damp8", [HEADS, 1], F32, isOutput=False)
        self.d_outb = p("outbr", [1, TF], F32, isOutput=False)
        self.d_out = p("outT", [S * TF, HOR], F32, isOutput=True)
        self.zmid = nc.dram_tensor("zmid", [S, N, D], F32)
        self.xtmid = nc.dram_tensor("xtmid", [S, TF, N], F32)

    # psum bank helper: tag-based reuse of the 8 banks
    def bank(self, i, shape=(128, 512)):
        tl = self.psp.tile(list(shape), F32, tag=f"bk{i}", name=f"bk{i}")
        return tl

    def build(self):
        nc = self.nc
        with ExitStack() as ctx:
            self.tc = ctx.enter_context(tile.TileContext(nc))
            tc = self.tc
            top = ctx.enter_context(tc.tile_pool(name="top", bufs=1))

            idn = top.tile([128, 128], F32, name="idn")
            nc.sync.dma_start(idn[:], self.d_idn[:])
            ones = top.tile([128, 128], F32, name="ones")
            nc.vector.memset(ones[:], 1.0)
            fmt = top.tile([128, 13 * 128], F32, name="fmt")
            nc.sync.dma_start(fmt[:], self.d_fm[:])
            imrt = top.tile([128, 13 * 128], F32R, name="imrt")
            nc.sync.dma_start(imrt[:], self.d_imr[:])
            pmt = top.tile([128, 128], F32, name="pmt")
            nc.sync.dma_start(pmt[:], self.d_pm[:])
            jrt = top.tile([128, 128], F32, name="jrt")
            nc.sync.dma_start(jrt[:], self.d_jr[:])
            self.fmt, self.imrt, self.pmt, self.jrt = fmt, imrt, pmt, jrt
            e8 = top.tile([HEADS, D], F32, name="e8")
            nc.sync.dma_start(e8[:], self.d_e8[:])
            w2d = top.tile([97, D], F32, name="w2d")
            nc.sync.dma_start(w2d[:], self.d_w2d[:])
            # rows pack: p32 = gpost|bpost (2x512); p64 = outb[7]
            rows = top.tile([128, 512], F32, name="rows")
            nc.sync.dma_start(rows[32:33, 0:512], self.d_gpost[:])
            nc.sync.dma_start(rows[64:65, 0:512], self.d_bpost[:])
            nc.sync.dma_start(rows[0:1, 0:TF], self.d_outb[:])
            # col pack: gpre(4) | bpre(4) | ffb1(16) | ffb2(4)
            cpk = top.tile([128, 28], F32, name="cpk")
            nc.sync.dma_start(cpk[:], self.d_cpkp[:])
            outw = top.tile([128, ND * TF], F32, name="outw")
            nc.sync.dma_start(outw[:], self.d_outwp[:])
            eps = top.tile([128, 1], F32, name="eps")
            nc.vector.memset(eps[:], 1e-5)
            self.epst = eps
            agg = top.tile([128, S * ND * HOR], F32, name="agg")
            nc.vector.memset(agg[:], 0.0)
            csd = top.tile([128, ND * HOR], F32, name="csd")

            self.idn, self.ones, self.rows, self.cpk = idn, ones, rows, cpk
            self.e8t, self.w2dt_, self.aggt, self.csdt = e8, w2d, agg, csd
            self.outwt = outw

            # ff_post g/b replicated over 128 partitions (built once)
            gbt = top.tile([128, D], F32, name="gbt")
            bbt = top.tile([128, D], F32, name="bbt")
            self.gbt, self.bbt = gbt, bbt

            with tc.tile_pool(name="ini", bufs=1) as ini, \
                    tc.tile_pool(name="inips", bufs=1, space="PSUM") as inips:
                self.psp = inips
                self._damp_cs(ini, inips)

            for l in range(L):
                last = l == L - 1
                with tc.tile_pool(name=f"lay{l}", bufs=1) as layp, \
                        tc.tile_pool(name=f"wk{l}", bufs=1) as wk, \
                        tc.tile_pool(name=f"ps{l}", bufs=1, space="PSUM") as psp:
                    self.psp = psp
                    lay = self._layer_consts(l, layp)
                    for s in range(S):
                        self._sample(l, s, lay, wk)
                    if last:
                        for s in range(S):
                            self._output(s, wk)

        nc.compile()
        return nc

    # ---------- dampening cumsum -> csd [128, ND*HOR] ----------
    def _damp_cs(self, ini, inips):  # inips: any psum pool
        nc = self.nc
        ones = self.ones
        dcol = ini.tile([HEADS, 1], F32, name="dcol")
        nc.sync.dma_start(dcol[:], self.d_damp[:])
        df = ini.tile([HEADS, 1], F32, name="dfsig")
        nc.scalar.activation(df[:], dcol[:], AF.Sigmoid)
        dfb = ini.tile([HEADS, HOR], F32, name="dfb")
        nc.scalar.activation(dfb[:], ones[0:HEADS, 0:HOR], AF.Identity,
                             scale=df[:, 0:1])
        zer = ini.tile([HEADS, HOR], F32, name="zer8")
        nc.vector.memset(zer[:], 0.0)
        dfp = ini.tile([HEADS, HOR], F32, name="dfp")
        nc.vector.tensor_tensor_scan(dfp[:], dfb[:], zer[:], 1.0,
                                     op0=ALU.mult, op1=ALU.add)
        cs8 = ini.tile([HEADS, HOR], F32, name="cs8")
        nc.vector.tensor_tensor_scan(cs8[:], ones[0:HEADS, 0:HOR], dfp[:], 0.0,
                                     op0=ALU.mult, op1=ALU.add)
        for dt in range(ND):
            pini = self.bank(6)
            nc.tensor.matmul(pini[:, 0:HOR], self.e8t[:, _sl(dt)], cs8[:],
                             start=True, stop=True)
            nc.scalar.copy(self.csdt[:, dt * HOR:(dt + 1) * HOR], pini[:, 0:HOR])
        # replicate ff_post g/b rows across partitions
        pgb = self.bank(7)
        nc.tensor.matmul(pgb[:], ones[32:33, 0:128], self.rows[32:33, 0:512],
                         start=True, stop=True)
        nc.scalar.copy(self.gbt[:], pgb[:])
        pbb = self.bank(6)
        nc.tensor.matmul(pbb[:], ones[64:65, 0:128], self.rows[64:65, 0:512],
                         start=True, stop=True)
        nc.scalar.copy(self.bbt[:], pbb[:])

    # ---------- per-layer constants ----------
    def _layer_consts(self, l, layp):
        nc = self.nc
        ones = self.ones
        last = l == L - 1
        lay = {"l": l, "last": last}

        win1 = layp.tile([128, ND * D], F32R, name="win1")
        nc.sync.dma_start(win1[:], self.d_win[l, :, :])
        wout1 = layp.tile([128, ND * D], F32R, name="wout1")
        nc.sync.dma_start(wout1[:], self.d_wout[l, :, :])
        win = [win1[:, kt * D:(kt + 1) * D] for kt in range(ND)]
        wout = [wout1[:, kt * D:(kt + 1) * D] for kt in range(ND)]

        # lrows: p0 = bout[512]; level biases as [TF,1] columns for ACT bias
        lrows = layp.tile([128, 512], F32, name="lrows")
        nc.sync.dma_start(lrows[0:1, 0:D], self.d_bout[l, :, :])
        bgcol = layp.tile([TF, 1], F32, name="bgcol")
        nc.sync.dma_start(bgcol[:], self.d_bg[l, :, :])
        bpcol = layp.tile([TF, 1], F32, name="bpcol")
        nc.sync.dma_start(bpcol[:], self.d_bp[l, :, :])
        # bout replicated across partitions for the Pool-engine bias add
        boutt = layp.tile([128, D], F32, name="boutt")
        pbo = self.psp.tile([128, D], F32, tag="bk2", name="pbo")
        nc.tensor.matmul(pbo[:], ones[0:1, 0:128], lrows[0:1, 0:D],
                         start=True, stop=True)
        nc.scalar.copy(boutt[:], pbo[:])


        # lcol pack [128, 16]: al(4) oma(4) init(4) bi(4); plus lv cols [7,1]
        lcol = layp.tile([128, 24], F32, name="lcol")
        al8 = layp.tile([HEADS, 1], F32, tag="al8t", name="al8")
        nc.sync.dma_start(al8[:], self.d_al8[l, :, :])
        al8s = layp.tile([HEADS, 1], F32, tag="al8s", name="al8s")
        nc.scalar.activation(al8s[:], al8[:], AF.Sigmoid)
        for dt in range(ND):
            pal = self.psp.tile([128, 1], F32, tag="bk0", name="pal")
            nc.tensor.matmul(pal[:], self.e8t[:, _sl(dt)], al8s[:],
                             start=True, stop=True)
            nc.scalar.copy(lcol[:, dt:dt + 1], pal[:])
        libi = layp.tile([128, 8], F32, tag="libi", name="libi")
        nc.sync.dma_start(libi[:], self.d_lini[l, :, :])
        nc.vector.tensor_copy(lcol[:, 8:16], libi[:, 0:8])
        for dt in range(ND):
            nc.vector.tensor_scalar(lcol[:, 4 + dt:5 + dt], lcol[:, dt:dt + 1],
                                    -1.0, 1.0, op0=ALU.mult, op1=ALU.add)
        # level alpha
        alv = layp.tile([1, 1], F32, tag="alvt", name="alv")
        nc.sync.dma_start(alv[:], self.d_alv[l, :, :])
        alvs = layp.tile([1, 1], F32, tag="alvst", name="alvs")
        nc.scalar.activation(alvs[:], alv[:], AF.Sigmoid)
        pv = self.psp.tile([TF, 1], F32, tag="bk1", name="palv")
        nc.tensor.matmul(pv[:], ones[0:1, 0:TF], alvs[:], start=True, stop=True)
        nc.scalar.copy(lcol[0:TF, 16:17], pv[:])
        nc.vector.tensor_scalar(lcol[0:TF, 17:18], lcol[0:TF, 16:17], -1.0, 1.0,
                                op0=ALU.mult, op1=ALU.add)

        # level weights [128, TF] x4 packed [128, 2*ND*TF], as fp32r
        lwf = layp.tile([128, 2 * ND * TF], F32, tag="lwf", name="lwf")
        nc.sync.dma_start(lwf[:], self.d_lvw[l, :, :])
        lw = layp.tile([128, 2 * ND * TF], F32R, name="lw")
        nc.vector.tensor_copy(lw[:], lwf[:])

        lay.update(win=win, wout=wout, lrows=lrows, lcol=lcol, lw=lw,
                   bgcol=bgcol, bpcol=bpcol, boutt=boutt)
        return lay

    # ---------- one sample through one layer ----------
    def _sample(self, l, s, lay, wk):
        nc = self.nc
        ones, idn = self.ones, self.idn
        last = lay["last"]
        agg = self.aggt

        def aggsl(dt):
            return self.aggt[:, (s * ND + dt) * HOR:(s * ND + dt + 1) * HOR]

        # --- z input: conv (l0) or reload (l1)
        z = [wk.tile([128, D], F32, tag=f"B1_{tt}", name=f"z{tt}")
             for tt in range(NT)]
        if l == 0:
            xsh = wk.tile([97, N], F32, tag="xd", name="xsh")
            xts = wk.tile([TF, N], F32, tag="xts", name="xts")
            nc.sync.dma_start(xts[:], self.d_xT[s * TF:(s + 1) * TF, :])
            nc.vector.memset(xsh[:], 0.0)
            nc.vector.tensor_copy(xsh[0:TF, 1:N], xts[:, 0:N - 1])
            nc.vector.tensor_copy(xsh[32:32 + TF, 0:N], xts[:, 0:N])
            nc.vector.tensor_copy(xsh[64:64 + TF, 0:N - 1], xts[:, 1:N])
            nc.vector.memset(xsh[96:97, :], 1.0)  # bias row (w2d row 96)
            for tt in range(NT):
                pz = self.bank(6 + tt % 2)
                nc.tensor.matmul(pz[:], xsh[:, _sl(tt)], self.w2dt_[:],
                                 start=True, stop=True)
                nc.scalar.copy(z[tt][:], pz[:])
        else:
            for tt in range(NT):
                nc.sync.dma_start(z[tt][:], self.zmid[s, _sl(tt), :])

        # ===== CT (8x128) rfft: stage 1 (DVE/Pool) -> G (B3), partials (B4)
        va, po = nc.vector, nc.gpsimd
        prt = [wk.tile([128, D], F32, tag=f"B4_{i}", name=f"prt{i}")
               for i in range(NT)]
        for u in range(4):
            va.tensor_add(prt[2 * u][:], z[u][:], z[u + 4][:])
            po.tensor_sub(prt[2 * u + 1][:], z[u][:], z[u + 4][:])
        a04, s04, a15, s15, a26, s26, a37, s37 = [p_[:] for p_ in prt]
        # G order: G0 G4 G1r G1i G2r G2i G3r G3i
        G = [wk.tile([128, D], F32, tag=f"B3_{i}", name=f"G{i}")
             for i in range(NT)]
        c0 = wk.tile([128, D], F32, tag="ct0", name="c0")
        c1t = wk.tile([128, D], F32, tag="ct1", name="c1t")
        va.tensor_add(c0[:], a04, a26)
        po.tensor_add(c1t[:], a15, a37)
        va.tensor_add(G[0][:], c0[:], c1t[:])
        va.tensor_sub(G[1][:], c0[:], c1t[:])
        po.tensor_sub(G[4][:], a04, a26)
        po.tensor_sub(G[5][:], a37, a15)
        qt = wk.tile([128, D], F32, tag="ct0", name="qt")
        va.tensor_sub(qt[:], s15, s37)
        pt_ = wk.tile([128, D], F32, tag="ct1", name="pt_")
        po.tensor_add(pt_[:], s15, s37)
        C1C = 0.7071067811865476
        va.scalar_tensor_tensor(G[2][:], qt[:], C1C, s04,
                                op0=ALU.mult, op1=ALU.add)
        va.scalar_tensor_tensor(G[3][:], pt_[:], -C1C, s26,
                                op0=ALU.mult, op1=ALU.subtract)
        va.scalar_tensor_tensor(G[6][:], qt[:], -C1C, s04,
                                op0=ALU.mult, op1=ALU.add)
        va.scalar_tensor_tensor(G[7][:], pt_[:], -C1C, s26,
                                op0=ALU.mult, op1=ALU.add)

        # ===== stage 2: X[k1] = [m,c] re|im (A2/X4); amp -> ampT [c, 640] (A1)
        fm = self.fmt
        xdt = F32R
        Xs = [wk.tile([128, 1024], xdt,
                      tag=(f"A2_{k1}" if k1 < 4 else "X4"), name=f"X{k1}")
              for k1 in range(5)]
        ampT = [wk.tile([128, 1024], F32, tag=f"A1_{ct}", name=f"ampT{ct}")
                for ct in range(ND)]
        FB = [0, 2, 5, 8, 11]

        def fmc(j):
            return fm[:, j * 128:(j + 1) * 128]

        for k1 in (0, 4, 2, 1, 3):
            bre = self.bank((2 * k1) % 6)
            bim = self.bank((2 * k1) % 6 + 1)
            b = FB[k1]
            if k1 in (0, 4):
                g = G[0] if k1 == 0 else G[1]
                nc.tensor.matmul(bre[:], fmc(b), g[:], start=True, stop=True)
                nc.tensor.matmul(bim[:], fmc(b + 1), g[:], start=True, stop=True)
            else:
                gr, gi = G[2 * k1], G[2 * k1 + 1]
                nc.tensor.matmul(bre[:], fmc(b), gr[:], start=True, stop=False)
                nc.tensor.matmul(bre[:], fmc(b + 1), gi[:], start=False, stop=True)
                nc.tensor.matmul(bim[:], fmc(b), gi[:], start=True, stop=False)
                nc.tensor.matmul(bim[:], fmc(b + 2), gr[:], start=False, stop=True)
            sq0 = wk.tile([128, D], F32, tag="sq0", name="sq0")
            nc.scalar.activation(sq0[:], bre[:], AF.Square)
            sq1 = wk.tile([128, D], F32, tag="sq1", name="sq1")
            nc.scalar.activation(sq1[:], bim[:], AF.Square)
            nc.scalar.copy(Xs[k1][:, 0:512], bre[:])
            nc.scalar.copy(Xs[k1][:, 512:1024], bim[:])
            amp = wk.tile([128, D], F32, tag=f"amp{k1 % 2}", name=f"amp{k1}")
            va.tensor_add(amp[:], sq0[:], sq1[:])
            pT = self.bank(6 + k1 % 2)
            for ct in range(ND):
                nc.tensor.transpose(pT[:, _sl(ct)], amp[:, _sl(ct)], idn[:])
            for ct in range(ND):
                nc.scalar.copy(ampT[ct][:, k1 * 128:(k1 + 1) * 128],
                               pT[:, _sl(ct)])
            if k1 in (0, 4):
                # mirror-duplicate zones via permutation matmul (exact copies)
                pM = self.bank(7 - k1 % 2)
                pmat = self.pmt if k1 == 0 else self.jrt
                for ct in range(ND):
                    nc.tensor.matmul(pM[:, _sl(ct)], amp[:, _sl(ct)], pmat[:],
                                     start=True, stop=True)
                off = 65 if k1 == 0 else 512 + 64
                lo = 65 if k1 == 0 else 64
                for ct in range(ND):
                    nc.scalar.copy(ampT[ct][:, off:(k1 * 128 + 128)],
                                   pM[:, ct * 128 + lo:(ct + 1) * 128])

        # ===== ranking: canon top-4 over [0:65] U [128:576]; in-place mask
        for ct in range(ND):
            t16 = wk.tile([128, 16], F32, tag=f"t16_{ct}", name="t16")
            va.max(t16[:, 0:8], ampT[ct][:, 0:65])
            va.max(t16[:, 8:16], ampT[ct][:, 128:576])
            top8 = wk.tile([128, 8], F32, tag=f"top8_{ct}", name="top8")
            va.max(top8[:], t16[:])
            va.tensor_scalar(ampT[ct][:, 0:640], ampT[ct][:, 0:640],
                             top8[:, 3:4], 0.0, op0=ALU.is_ge, op1=ALU.add)

        # ===== mask transpose per k1; apply to X
        for k1 in range(5):
            pM = self.bank(6 + k1 % 2)
            for ct in range(ND):
                nc.tensor.transpose(pM[:, _sl(ct)],
                                    ampT[ct][:, k1 * 128:(k1 + 1) * 128], idn[:])
            va.tensor_mul(Xs[k1][:, 0:512], Xs[k1][:, 0:512], pM[:])
            va.tensor_mul(Xs[k1][:, 512:1024], Xs[k1][:, 512:1024], pM[:])

        # ===== inverse: 16 matmuls -> 8 banks -> W (B4)
        imt = self.imrt
        W = [wk.tile([128, D], F32, tag=f"B4_{i}", name=f"W{i}")
             for i in range(NT)]

        def imc(j):
            return imt[:, j * 128:(j + 1) * 128]

        pb = [self.bank(i) for i in range(8)]
        nc.tensor.matmul(pb[0][:], imc(0), Xs[0][:, 0:512],
                         start=True, stop=False)
        nc.tensor.matmul(pb[0][:], imc(1), Xs[0][:, 512:1024],
                         start=False, stop=True)
        nc.tensor.matmul(pb[1][:], imc(11), Xs[4][:, 0:512],
                         start=True, stop=False)
        nc.tensor.matmul(pb[1][:], imc(12), Xs[4][:, 512:1024],
                         start=False, stop=True)
        for k1 in (1, 2, 3):
            b = FB[k1]
            br, bi = pb[2 * k1], pb[2 * k1 + 1]
            nc.tensor.matmul(br[:], imc(b), Xs[k1][:, 0:512],
                             start=True, stop=False)
            nc.tensor.matmul(br[:], imc(b + 2), Xs[k1][:, 512:1024],
                             start=False, stop=True)
            nc.tensor.matmul(bi[:], imc(b + 1), Xs[k1][:, 0:512],
                             start=True, stop=False)
            nc.tensor.matmul(bi[:], imc(b), Xs[k1][:, 512:1024],
                             start=False, stop=True)
        for i in range(8):
            nc.scalar.copy(W[i][:], pb[i][:])

        # ===== combine -> lp (B3); W order: W0 P4 Z1r Z1i Z2r Z2i Z3r Z3i
        lp = [wk.tile([128, D], F32, tag=f"B3_{tt}", name=f"lp{tt}")
              for tt in range(NT)]
        A_ = wk.tile([128, D], F32, tag="ct0", name="A_")
        Bm = wk.tile([128, D], F32, tag="ct1", name="Bm")
        va.tensor_add(A_[:], W[0][:], W[1][:])
        po.tensor_sub(Bm[:], W[0][:], W[1][:])
        R13p = wk.tile([128, D], F32, tag="sq0", name="R13p")
        va.tensor_add(R13p[:], W[2][:], W[6][:])
        R13m = wk.tile([128, D], F32, tag="sq1", name="R13m")
        po.tensor_sub(R13m[:], W[2][:], W[6][:])
        I13p = wk.tile([128, D], F32, tag="amp0", name="I13p")
        va.tensor_add(I13p[:], W[3][:], W[7][:])
        I13m = wk.tile([128, D], F32, tag="amp1", name="I13m")
        po.tensor_sub(I13m[:], W[3][:], W[7][:])
        va.tensor_add(W[0][:], A_[:], W[4][:])       # E0
        po.tensor_sub(W[1][:], Bm[:], W[5][:])       # E1
        va.tensor_sub(A_[:], A_[:], W[4][:])         # E2
        po.tensor_add(Bm[:], Bm[:], W[5][:])         # E3
        va.tensor_sub(W[2][:], R13m[:], I13p[:])     # q1
        po.tensor_add(W[3][:], R13m[:], I13p[:])     # q3
        va.tensor_add(lp[0][:], W[0][:], R13p[:])
        po.tensor_sub(lp[4][:], W[0][:], R13p[:])
        va.scalar_tensor_tensor(lp[1][:], W[2][:], C1C, W[1][:],
                                op0=ALU.mult, op1=ALU.add)
        va.scalar_tensor_tensor(lp[5][:], W[2][:], -C1C, W[1][:],
                                op0=ALU.mult, op1=ALU.add)
        va.tensor_sub(lp[2][:], A_[:], I13m[:])
        po.tensor_add(lp[6][:], A_[:], I13m[:])
        va.scalar_tensor_tensor(lp[3][:], W[3][:], -C1C, Bm[:],
                                op0=ALU.mult, op1=ALU.add)
        va.scalar_tensor_tensor(lp[7][:], W[3][:], C1C, Bm[:],
                                op0=ALU.mult, op1=ALU.add)
        z2 = [wk.tile([128, D], F32, tag=f"B4_{tt}", name=f"z2_{tt}")
              for tt in range(NT)]
        for tt in range(NT):
            eng = va if tt % 2 == 0 else po
            eng.tensor_sub(z2[tt][:], z[tt][:], lp[tt][:])

        # --- lpT [ND][128, N] (tag A2) + extrap + perT; then free
        lpT = [wk.tile([128, N], F32R, tag=f"A2_{dt}", name=f"lpT{dt}")
               for dt in range(ND)]
        for dt in range(ND):
            for h in range(2):
                pT = self.bank(dt % 2)
                for q in range(4):
                    nc.tensor.transpose(pT[:, _sl(q)], lp[h * 4 + q][:, _sl(dt)],
                                        idn[:])
                nc.scalar.copy(lpT[dt][:, _hh(h)], pT[:])
            nc.vector.tensor_add(aggsl(dt), aggsl(dt), lpT[dt][:, 0:HOR])
        perT = wk.tile([TF, N], F32, tag="perT", name="perT")
        for h in range(2):
            pp = self.bank(2)
            for kt in range(ND):
                nc.tensor.matmul(pp[0:TF, :], lay["lw"][:, (ND + kt) * TF:(ND + kt + 1) * TF],
                                 lpT[kt][:, _hh(h)], start=(kt == 0),
                                 stop=(kt == ND - 1))
            nc.scalar.activation(perT[:, _hh(h)], pp[0:TF, :], AF.Identity,
                                 bias=lay["bpcol"][:, 0:1])

        # --- z2T (tag A2 reuse after lpT dead)
        z2T = [wk.tile([128, N], F32R, tag=f"A2_{dt}", name=f"z2T{dt}")
               for dt in range(ND)]
        for dt in range(ND):
            for h in range(2):
                pT = self.bank(dt % 2)
                for q in range(4):
                    nc.tensor.transpose(pT[:, _sl(q)], z2[h * 4 + q][:, _sl(dt)],
                                        idn[:])
                nc.vector.tensor_copy(z2T[dt][:, _hh(h)], pT[:])

        # --- win GEMM -> xinT (tag A1 reuse: filt dead)
        xinT = [wk.tile([128, N], F32, tag=f"A1_{dt}", name=f"xinT{dt}")
                for dt in range(ND)]
        for dt in range(ND):
            for h in range(2):
                px = self.bank(4 + dt % 2)
                for kt in range(ND):
                    nc.tensor.matmul(px[:], lay["win"][kt][:, _sl(dt)],
                                     z2T[kt][:, _hh(h)],
                                     start=(kt == 0), stop=(kt == ND - 1))
                nc.scalar.copy(xinT[dt][:, _hh(h)], px[:])

        # --- xd -> scan -> sT (tag A2 reuse: z2T dead)
        sT = [wk.tile([128, N], F32R, tag=f"A2_{dt}", name=f"sT{dt}")
              for dt in range(ND)]
        lc = lay["lcol"]
        for dt in range(ND):
            xd = wk.tile([128, N], F32, tag="xd", name="xd")
            nc.vector.tensor_sub(xd[:, 1:N], xinT[dt][:, 1:N], xinT[dt][:, 0:N - 1])
            nc.vector.tensor_scalar_add(xd[:, 0:1], xinT[dt][:, 0:1],
                                        lc[:, 12 + dt:13 + dt])
            if OMA_BCAST:
                omab_ap = lc[:, 4 + dt:5 + dt].broadcast_to([128, N])
            else:
                omab = wk.tile([128, N], F32, tag="omab", name="omab")
                nc.vector.memset(omab[:], 1.0)
                nc.vector.tensor_scalar_mul(omab[:], omab[:], lc[:, 4 + dt:5 + dt])
                omab_ap = omab[:]
            nc.vector.tensor_tensor_scan(sT[dt][:], omab_ap, xd[:],
                                         lc[:, 8 + dt:9 + dt],
                                         op0=ALU.mult, op1=ALU.add)

        # --- wout GEMM -> lg [t,d] (tag B2 reuse: filtT dead) (+ z3 if l0)
        lg = [wk.tile([128, D], F32, tag=f"B2_{tt}", name=f"lg{tt}")
              for tt in range(NT)]
        for tt in range(NT):
            pg = self.bank(tt % 2)
            for kt in range(ND):
                nc.tensor.matmul(pg[:], sT[kt][:, _sl(tt)], lay["wout"][kt],
                                 start=(kt == 0), stop=(kt == ND - 1))
            nc.vector.tensor_add(lg[tt][:], pg[:], lay["boutt"][:])
            if not last:
                # z3 overwrites z (tag B1): z dead after z2
                nc.vector.tensor_sub(z[tt][:], z2[tt][:], lg[tt][:])
        z3 = z

        # --- lgT via transposes (tag A1 reuse: xinT dead)
        lgT = [wk.tile([128, N], F32R, tag=f"A1_{dt}", name=f"lgT{dt}")
               for dt in range(ND)]
        for dt in range(ND):
            for h in range(2):
                pT = self.bank(2 + dt % 2)
                for q in range(4):
                    nc.tensor.transpose(pT[:, _sl(q)], lg[h * 4 + q][:, _sl(dt)],
                                        idn[:])
                nc.scalar.copy(lgT[dt][:, _hh(h)], pT[:])
            # damp: agg += lg_last * csd
            nc.vector.scalar_tensor_tensor(
                aggsl(dt), self.csdt[:, dt * HOR:(dt + 1) * HOR],
                lgT[dt][:, N - 1:N], aggsl(dt), op0=ALU.mult, op1=ALU.add)

        # --- level: grT; scans update xtmid
        grT = wk.tile([TF, N], F32, tag="grT", name="grT")
        for h in range(2):
            pgr = self.bank(4)
            for kt in range(ND):
                nc.tensor.matmul(pgr[0:TF, :], lay["lw"][:, kt * TF:(kt + 1) * TF],
                                 lgT[kt][:, _hh(h)], start=(kt == 0),
                                 stop=(kt == ND - 1))
            nc.scalar.activation(grT[:, _hh(h)], pgr[0:TF, :], AF.Identity,
                                 bias=lay["bgcol"][:, 0:1])

        xts2 = wk.tile([TF, N], F32, tag="xts", name="xts2")
        if l == 0:
            nc.sync.dma_start(xts2[:], self.d_xT[s * TF:(s + 1) * TF, :])
        else:
            nc.sync.dma_start(xts2[:], self.xtmid[s, :, :])
        v = wk.tile([TF, N], F32, tag="lvv", name="lvv")
        nc.vector.tensor_sub(v[:], xts2[:], perT[:])
        nc.vector.tensor_scalar_mul(v[:], v[:], lc[0:TF, 16:17])
        if OMA_BCAST:
            omlv_ap = lc[0:TF, 17:18].broadcast_to([TF, N])
        else:
            omlv = wk.tile([TF, N], F32, tag="omlv", name="omlv")
            nc.vector.memset(omlv[:], 1.0)
            nc.vector.tensor_scalar_mul(omlv[:], omlv[:], lc[0:TF, 17:18])
            omlv_ap = omlv[:]
        pt = wk.tile([TF, N], F32, tag="lvp", name="lvp")
        nc.vector.tensor_tensor_scan(pt[:], omlv_ap, v[:], 0.0,
                                     op0=ALU.mult, op1=ALU.add)
        gt = wk.tile([TF, N], F32, tag="lvv", name="lvg")
        nc.vector.tensor_tensor_scan(gt[:], omlv_ap, grT[:], 0.0,
                                     op0=ALU.mult, op1=ALU.add)
        xnew = wk.tile([TF, N], F32, tag="grT", name="xnew")
        nc.vector.tensor_add(xnew[:], pt[:], gt[:])
        nc.sync.dma_start(self.xtmid[s, :, :], xnew[:])

        # --- FF (layer 0 only), then spill z4
        if not last:
            z4 = self._ff(s, z3, wk)
            for tt in range(NT):
                nc.sync.dma_start(self.zmid[s, _sl(tt), :], z4[tt][:])

    # ---------- LN stats ----------
    def _ln_stats(self, zset, wk, tagp):
        nc = self.nc
        st = wk.tile([128, 8 * NT], F32, tag=f"st{tagp}", name=f"st{tagp}")
        mu8 = st[:, 0:NT]
        s28 = st[:, NT:2 * NT]
        scr = wk.tile([128, D], F32, tag="lnscr", name="lnscr")
        for tt in range(NT):
            nc.vector.tensor_reduce(st[:, tt:tt + 1], zset[tt][:],
                                    mybir.AxisListType.X, op=ALU.add)
            nc.scalar.activation(scr[:], zset[tt][:], AF.Square,
                                 accum_out=st[:, NT + tt:NT + tt + 1])
        mun = st[:, 2 * NT:3 * NT]
        nc.vector.tensor_scalar_mul(mun, mu8, 1.0 / D)
        ex2 = st[:, 3 * NT:4 * NT]
        nc.vector.tensor_scalar_mul(ex2, s28, 1.0 / D)
        musq = st[:, 4 * NT:5 * NT]
        nc.scalar.activation(musq, mun, AF.Square)
        var = st[:, 5 * NT:6 * NT]
        nc.vector.tensor_sub(var, ex2, musq)
        sd = st[:, 6 * NT:7 * NT]
        nc.scalar.activation(sd, var, AF.Sqrt, bias=self.epst[:, 0:1])
        rs = st[:, 7 * NT:8 * NT]
        nc.vector.reciprocal(rs, sd)
        nmurs = st[:, 4 * NT:5 * NT]  # overwrite musq slot
        nc.vector.tensor_mul(nmurs, mun, rs)
        nc.vector.tensor_scalar_mul(nmurs, nmurs, -1.0)
        return rs, nmurs

    # ---------- FF block ----------
    def _ff(self, s, z3, wk):
        nc = self.nc
        ones, idn = self.ones, self.idn
        rows, cpk = self.rows, self.cpk
        rs, nmurs = self._ln_stats(z3, wk, "pre")
        # h = (z3-mu)*rs, overwrite z3 tiles in place via scratch
        h_ = [wk.tile([128, D], F32, tag=f"B2_{tt}", name=f"h{tt}")
              for tt in range(NT)]
        for tt in range(NT):
            nc.scalar.activation(h_[tt][:], z3[tt][:], AF.Identity,
                                 scale=rs[:, tt:tt + 1], bias=nmurs[:, tt:tt + 1])
        hT = [wk.tile([128, N], F32, tag=f"A2_{dt}", name=f"hT{dt}")
              for dt in range(ND)]
        for dt in range(ND):
            for h in range(2):
                pT = self.bank(dt % 2)
                for q in range(4):
                    nc.tensor.transpose(pT[:, _sl(q)], h_[h * 4 + q][:, _sl(dt)],
                                        idn[:])
                nc.scalar.copy(hT[dt][:, _hh(h)], pT[:])
        znT = [wk.tile([128, N], F32R, tag=f"A1_{dt}", name=f"znT{dt}")
               for dt in range(ND)]
        for dt in range(ND):
            nc.scalar.activation(znT[dt][:], hT[dt][:], AF.Identity,
                                 scale=cpk[:, dt:dt + 1], bias=cpk[:, 4 + dt:5 + dt])

        yT = [wk.tile([128, N], F32, tag=f"A2_{dt}", name=f"yT{dt}")
              for dt in range(ND)]
        for h in range(2):
            pzf = [self.bank(dt) for dt in range(ND)]
            for m in range(NM):
                w1m = wk.tile([128, D], F32R, tag=f"w1m{m % 2}", name="w1m")
                nc.sync.dma_start(w1m[:], self.d_ffw1p[:, m * D:(m + 1) * D])
                w2m = wk.tile([128, D], F32R, tag=f"w2m{m % 2}", name="w2m")
                nc.sync.dma_start(w2m[:], self.d_ffw2p[:, m * D:(m + 1) * D])
                ph = self.bank(4 + m % 2)
                for kt in range(ND):
                    nc.tensor.matmul(
                        ph[:], w1m[:, kt * 128:(kt + 1) * 128],
                        znT[kt][:, _hh(h)],
                        start=(kt == 0), stop=(kt == ND - 1))
                sig = wk.tile([128, 512], F32R, tag=f"sig{m % 2}", name="sig")
                nc.scalar.activation(sig[:], ph[:], AF.Sigmoid,
                                     bias=cpk[:, 8 + m:9 + m])
                for dt in range(ND):
                    nc.tensor.matmul(pzf[dt][:], w2m[:, dt * 128:(dt + 1) * 128],
                                     sig[:], start=(m == 0), stop=(m == NM - 1))
            for dt in range(ND):
                nc.vector.scalar_tensor_tensor(yT[dt][:, _hh(h)], pzf[dt][:],
                                               cpk[:, 24 + dt:25 + dt],
                                               znT[dt][:, _hh(h)].bitcast(F32),
                                               op0=ALU.add, op1=ALU.add)

        y = [wk.tile([128, D], F32, tag=f"B4_{tt}", name=f"y{tt}")
             for tt in range(NT)]
        for tt in range(NT):
            pT = self.bank(2 + tt % 2)
            for dt in range(ND):
                nc.tensor.transpose(pT[:, _sl(dt)], yT[dt][:, _sl(tt)], idn[:])
            nc.scalar.copy(y[tt][:], pT[:])

        rs2, nmurs2 = self._ln_stats(y, wk, "post")
        z4 = [wk.tile([128, D], F32, tag=f"B1_{tt}", name=f"z4_{tt}")
              for tt in range(NT)]
        scr2 = wk.tile([128, D], F32, tag="lnscr", name="scr2")
        for tt in range(NT):
            nc.scalar.activation(scr2[:], y[tt][:], AF.Identity,
                                 scale=rs2[:, tt:tt + 1], bias=nmurs2[:, tt:tt + 1])
            nc.vector.tensor_mul(z4[tt][:], scr2[:], self.gbt[:])
            nc.vector.tensor_add(z4[tt][:], z4[tt][:], self.bbt[:])
        return z4

    # ---------- output head ----------
    def _output(self, s, wk):
        nc = self.nc
        ones = self.ones
        po = self.bank(7)
        for kt in range(ND):
            nc.tensor.matmul(po[0:TF, 0:HOR], self.outwt[:, kt * TF:(kt + 1) * TF],
                             self.aggt[:, (s * ND + kt) * HOR:(s * ND + kt + 1) * HOR],
                             start=(kt == 0), stop=False)
        nc.tensor.matmul(po[0:TF, 0:HOR], self.rows[0:1, 0:TF],
                         ones[0:1, 0:HOR], start=False, stop=True)
        xfin = wk.tile([TF, N], F32, tag="xts", name="xfin")
        nc.sync.dma_start(xfin[:], self.xtmid[s, :, :])
        oT = wk.tile([TF, HOR], F32, tag="oT", name="oT")
        nc.vector.tensor_scalar_add(oT[:], po[0:TF, 0:HOR], xfin[:, N - 1:N])
        nc.sync.dma_start(self.d_out[s * TF:(s + 1) * TF, :], oT[:])


def _get_nc():
    if "nc" not in _CACHE:
        _CACHE["nc"] = K().build()
    return _CACHE["nc"]


def _common_maps(inputs, w2d, cts, e8):
    ffw1 = np.asarray(inputs["ff_w1"], np.float32)
    ffw2 = np.asarray(inputs["ff_w2"], np.float32)
    fm, im, pm, jr = cts
    return dict(
        w2d=w2d,
        fmats=fm, imats=im, imatsr=im, permm=pm, permj=jr,
        idn=np.eye(128, dtype=np.float32),
        e8=e8,
        win=(np.asarray(inputs["mhesa_win"], np.float32) * _al_d(inputs)[:, None, :])
        .reshape(L, 4, 128, D).transpose(0, 2, 1, 3).reshape(L, 128, -1).copy(),
        wout=np.asarray(inputs["mhesa_wout"], np.float32)
        .reshape(L, 4, 128, D).transpose(0, 2, 1, 3).reshape(L, 128, -1).copy(),
        boutr=np.asarray(inputs["mhesa_bout"], np.float32).reshape(L, 1, D),
        alpha8=np.asarray(inputs["mhesa_alpha"], np.float32).reshape(L, HEADS, 1),
        ffw1p=_pack_w1(ffw1),
        ffw2p=_pack_w2(ffw2),
        cpkp=np.concatenate([
            np.asarray(inputs["ff_pre_g"], np.float32).reshape(4, 128).T,
            np.asarray(inputs["ff_pre_b"], np.float32).reshape(4, 128).T,
            np.asarray(inputs["ff_b1"], np.float32).reshape(16, 128).T,
            np.asarray(inputs["ff_b2"], np.float32).reshape(4, 128).T,
        ], axis=1),
        outwp=np.asarray(inputs["out_w"], np.float32)
        .reshape(4, 128, TF).transpose(1, 0, 2).reshape(128, -1).copy(),
        lvwp2=np.concatenate([
            np.asarray(inputs["level_wg"], np.float32)
            .reshape(L, 4, 128, TF).transpose(0, 2, 1, 3).reshape(L, 128, -1),
            np.asarray(inputs["level_wp"], np.float32)
            .reshape(L, 4, 128, TF).transpose(0, 2, 1, 3).reshape(L, 128, -1),
        ], axis=2),
        linip=np.concatenate([
            np.asarray(inputs["mhesa_init"], np.float32)
            .reshape(L, 4, 128).transpose(0, 2, 1),
            (_al_d(inputs) * (np.asarray(inputs["mhesa_bin"], np.float32)
                              - np.asarray(inputs["mhesa_init"], np.float32)
                              .reshape(L, D)))
            .reshape(L, 4, 128).transpose(0, 2, 1),
        ], axis=2),
        gpostr=np.asarray(inputs["ff_post_g"], np.float32).reshape(1, D),
        bpostr=np.asarray(inputs["ff_post_b"], np.float32).reshape(1, D),
        lvbg=np.asarray(inputs["level_bg"], np.float32).reshape(L, TF, 1),
        lvbp=np.asarray(inputs["level_bp"], np.float32).reshape(L, TF, 1),
        lvalpha=np.asarray(inputs["level_alpha"], np.float32).reshape(L, 1, 1),
        damp8=np.asarray(inputs["dampen_factor"], np.float32).reshape(HEADS, 1),
        outw=np.asarray(inputs["out_w"], np.float32),
        outbr=np.asarray(inputs["out_b"], np.float32).reshape(1, TF),
    )


def _kernel_impl(inputs, runner):
    x = np.asarray(inputs["x"], np.float32)
    assert (x.shape[0], x.shape[1], x.shape[2]) == (32, N, TF)
    assert int(inputs["forecast_horizon"]) == HOR
    cts = _ct_consts()
    conv_w = np.asarray(inputs["conv_w"], np.float32)
    w2d = _build_w2d(conv_w, np.asarray(inputs["conv_b"], np.float32))
    e8 = np.repeat(np.eye(HEADS, dtype=np.float32), DH, axis=1)
    nc = _get_nc()
    common = _common_maps(inputs, w2d, cts, e8)
    in_maps = []
    for c in range(NCORES):
        xs = x[c * S:(c + 1) * S]
        xT = xs.transpose(0, 2, 1).reshape(S * TF, N).copy()
        in_maps.append(dict(common, xT=xT))
    res = runner(nc, in_maps)
    out = np.zeros((x.shape[0], HOR, TF), np.float32)
    for c in range(NCORES):
        oT = res.results[c]["outT"].reshape(S, TF, HOR)
        out[c * S:(c + 1) * S] = oT.transpose(0, 2, 1)
    return out, res


def kernel(**inputs):
    out, _ = _kernel_impl(
        inputs,
        lambda nc, im: run_bass_kernel_spmd(nc, im, list(range(NCORES))))
    return out


def kernel_traced(**inputs):
    """Like kernel() but with NTFF profiling; returns (out, BassKernelResults)."""
    return _kernel_impl(
        inputs,
        lambda nc, im: run_bass_kernel_spmd(nc, im, list(range(NCORES)),
                                            trace=True))

